# revision 19
# baseline (speedup 1.0000x reference)
"""DarkChannel kernel for Trainium2: channel-min + 15x15 separable min-pool.

Full input img [16, 3, 1024, 1024] f32 -> output [16, 1, 1024, 1024] f32.
Batch-sharded across 8 NeuronCores (2 images per core).

Internals run in bf16 (tolerance 2e-2 >> bf16 rounding 2^-9): halves SBUF
traffic, enables the DVE 2x_1p mode, and quarters PE matmul cost vs f32.
Loads are split over all three DMA queues: c0 on sync and c1 on scalar
stay f32 (HWDGE cannot cast); c2 rides the gpsimd SWDGE queue casting
f32->bf16 in the DMA datapath. Output is written bf16 (gpsimd queue) and
upconverted on the host.

Per-core pipeline over 9 row-blocks (114 output rows each):
  - load 128 rows (7-row halo re-read) x 3 ch x 2 img; edge halos use a
    large finite fill (+inf would NaN-poison the shift matmuls via 0*inf)
  - channel min: 2 DVE tensor_tensor mins (bf16 2x) into padded A
  - h-pass: window-15 min as a 4-step doubling tree on DVE; shifts are
    free-dim AP offsets (TENSOR_TENSOR runs only on DVE on this compiler)
  - v-pass: doubling tree {1,2,4,7}; row shifts are shift-matrix matmuls
    on PE (bf16 -> PSUM f32), ACT copies each shift to SBUF bf16, DVE
    mins at 2x (Pool cannot access PSUM on TRN2)

The emission is software-pipelined with a 1-block skew: block b's
channel-min/h-pass ops are interleaved between block b-1's v-stage mins
so the in-order DVE queue always has ready work while the PE->ACT->DVE
v-chain round-trips complete.
"""
import sys
sys.path.insert(0, '/opt/trn_rl_repo')

import numpy as np

import concourse.bacc as bacc_mod
import concourse.mybir as mybir
from concourse.tile import TileContext
from concourse import bass_utils

F32 = mybir.dt.float32
BF16 = mybir.dt.bfloat16
MIN = mybir.AluOpType.min
BIG = 3e38   # large finite; +inf would NaN-poison the shift matmuls (0*inf)

H = 1024
W = 1024
C = 3
NIMG = 2              # images per core
N_CORES = 8
RBLK = 114            # output rows per block
NBLK = 9
LPAD = 8              # left pad; image at [8, 1032)
IPH = 1056            # per-image pitch (2112 B, 32B aligned)
HALO = 14

_cache = {}


def _tt(eng, out, in0, in1, op=MIN):
    return eng.add_instruction(mybir.InstTensorTensor(
        name=eng.bass.get_next_instruction_name(), op=op,
        ins=[eng.lower_ap(in0), eng.lower_ap(in1)],
        outs=[eng.lower_ap(out)]))


def _build():
    nc = bacc_mod.Bacc("TRN2", target_bir_lowering=False, debug=False,
                       num_devices=N_CORES)
    img = nc.dram_tensor("img", [NIMG, C, H, W], F32, kind="ExternalInput")
    out = nc.dram_tensor("out", [NIMG, 1, H, W], BF16, kind="ExternalOutput")

    with TileContext(nc) as tc:
        with tc.tile_pool(name="const", bufs=1) as cpool, \
             tc.tile_pool(name="cin", bufs=4) as chpool, \
             tc.tile_pool(name="work", bufs=2) as wpool, \
             tc.tile_pool(name="curp", bufs=6) as cpool3, \
             tc.tile_pool(name="cbp", bufs=2) as cbpool, \
             tc.tile_pool(name="psum", bufs=2, space="PSUM") as ppool:

            # ---- constants ----
            wmats = {}
            for d in (1, 2, 4, 7):
                wm = cpool.tile([128, 128], BF16, tag=f"wm{d}")
                nc.gpsimd.memset(wm[:], 1.0)
                # lhsT W[k, m] = 1 iff m == k - d  => out[m] = in[m + d]
                nc.gpsimd.affine_select(
                    out=wm[:], in_=wm[:],
                    compare_op=mybir.AluOpType.is_equal, fill=0.0,
                    base=d, channel_multiplier=-1, pattern=[[1, 128]])
                wmats[d] = wm

            # block b's A tile holds cmin of abs rows [r0-7, r0+rout+7);
            # loads clamp to the image; missing edge rows become BIG fills
            state = {}   # per-block tiles

            def emit_loads(b):
                r0 = b * RBLK
                rout = min(RBLK, H - r0)
                lo, hi = r0 - 7, r0 + rout + 7
                src_lo, src_hi = max(lo, 0), min(hi, H)
                dst0 = src_lo - lo
                n = src_hi - src_lo
                # c0/c1 stay f32 on the two HWDGE queues; c2 is cast to
                # bf16 by the (slower, single) SWDGE queue
                ct = chpool.tile([128, 2, NIMG, W], F32, tag="ct")
                c2 = chpool.tile([128, NIMG, W], BF16, tag="c2")
                for c in range(2):
                    for i in range(NIMG):
                        eng = nc.sync if c == 0 else nc.scalar
                        eng.dma_start(
                            out=ct[dst0:dst0 + n, c, i, :],
                            in_=img[i, c, src_lo:src_hi, :])
                for i in range(NIMG):
                    nc.gpsimd.dma_start(
                        out=c2[dst0:dst0 + n, i, :],
                        in_=img[i, 2, src_lo:src_hi, :])
                state[b] = {'ct': ct, 'c2': c2}

            def emit_conv(b):
                # Pool converts c0/c1 f32 -> bf16 (affine_select with an
                # always-true predicate is the only dtype-converting copy
                # this compiler accepts on the Pool engine), so min01 can
                # run on DVE in the 2x bf16 mode
                ct = state[b]['ct']
                cb = cbpool.tile([128, 2, NIMG, W], BF16, tag="cb")
                state[b]['cb'] = cb
                for c in range(2):
                    for i in range(NIMG):
                        nc.gpsimd.affine_select(
                            out=cb[:, c, i], in_=ct[:, c, i],
                            compare_op=mybir.AluOpType.is_ge, fill=0.0,
                            base=0, channel_multiplier=0, pattern=[[0, W]])

            def emit_ch1(b):
                A = wpool.tile([128, NIMG, IPH], BF16, tag="A")
                r0 = b * RBLK
                rout = min(RBLK, H - r0)
                hi = r0 + rout + 7
                nvalid = min(hi, H) - (r0 - 7)
                nc.gpsimd.memset(A[:, :, 0:LPAD], BIG)
                nc.gpsimd.memset(A[:, :, LPAD + W:LPAD + W + 16], BIG)
                cb = state[b]['cb']
                if hi > H:
                    nc.gpsimd.memset(A[96:128, :, :], BIG)
                    Amid = A[0:nvalid, :, LPAD:LPAD + W]
                    cbv = cb[0:nvalid]
                    c2v = state[b]['c2'][0:nvalid]
                else:
                    Amid = A[:, :, LPAD:LPAD + W]
                    cbv = cb
                    c2v = state[b]['c2']
                state[b].update(A=A, Amid=Amid, c2v=c2v)
                _tt(nc.vector, Amid, cbv[:, 0], cbv[:, 1])

            def emit_ch2(b):
                s = state[b]
                # per-image 2D ops merge to one contiguous free dim, which
                # the DVE bf16 2x mode requires (the 1056-pitch 3D form
                # runs at 1x)
                np_ = s['Amid'].shape[0]
                A, c2 = s['A'], s['c2']
                for i in range(NIMG):
                    _tt(nc.vector, A[0:np_, i, LPAD:LPAD + W],
                        A[0:np_, i, LPAD:LPAD + W], c2[0:np_, i, :])
                if b == 0:
                    nc.gpsimd.memset(s['A'][0:7, :, :], BIG)

            def emit_h(b, step):
                s = state[b]
                A = s['A']
                if step == 1:
                    Bv = wpool.tile([128, NIMG, IPH], BF16, tag="B")
                    s['B'] = Bv
                    for i in range(NIMG):
                        _tt(nc.vector, Bv[:, i, 0:1040], A[:, i, 0:1040],
                            A[:, i, 1:1041])
                elif step == 2:
                    Cv = wpool.tile([128, NIMG, IPH], BF16, tag="C")
                    s['C'] = Cv
                    for i in range(NIMG):
                        _tt(nc.vector, Cv[:, i, 0:1038],
                            s['B'][:, i, 0:1038], s['B'][:, i, 2:1040])
                elif step == 3:
                    for i in range(NIMG):
                        _tt(nc.vector, s['B'][:, i, 0:1034],
                            s['C'][:, i, 0:1034], s['C'][:, i, 4:1038])
                else:
                    cur = cpool3.tile([128, NIMG, W], BF16, tag="cur")
                    s['cur'] = cur
                    for i in range(NIMG):
                        _tt(nc.vector, cur[:, i, :], s['B'][:, i, 1:1025],
                            s['B'][:, i, 8:1032])

            def emit_vstage(b, d):
                cur = state[b]['cur']
                ps = ppool.tile([128, NIMG, 1024], F32, tag="ps")
                for i in range(NIMG):
                    for h2 in range(2):
                        nc.tensor.matmul(
                            ps[:, i, h2 * 512:(h2 + 1) * 512],
                            wmats[d][:],
                            cur[:, i, h2 * 512:(h2 + 1) * 512],
                            start=True, stop=True)
                sb = wpool.tile([128, NIMG, W], BF16, tag=f"sh{d}")
                nc.scalar.copy(out=sb[:, :, :], in_=ps[:, :, :])
                _tt(nc.vector, cur[:, :, :], cur[:, :, :], sb[:, :, :])

            def emit_outs(b):
                cur = state[b]['cur']
                r0 = b * RBLK
                rout = min(RBLK, H - r0)
                # outs ride the gpsimd queue so they don't block the
                # sync queue (halo copies + c0 loads) while waiting
                for i in range(NIMG):
                    nc.gpsimd.dma_start(out=out[i, 0, r0:r0 + rout, :],
                                        in_=cur[0:rout, i, :])

            # ---- software-pipelined emission, 4-slot-deep v-chain ----
            # Stage d of block b runs in slot b+k (k: d1->1, d2->2, d4->3,
            # d7->4), so every v-stage's dependency (the previous stage's
            # min) resolved a full slot earlier: PE/ACT/DVE start each slot
            # with only ready work and the 3-engine round-trip latency is
            # fully hidden. outs trail one further slot so they never block
            # a DMA queue head.
            for b in range(min(3, NBLK)):
                emit_loads(b)
            emit_conv(0)
            for s in range(NBLK + 5):
                if 0 <= s - 5 < NBLK:
                    emit_outs(s - 5)
                if 1 <= s + 1 < NBLK:
                    emit_conv(s + 1)
                if s < NBLK:
                    emit_ch1(s)
                    emit_ch2(s)
                if 0 <= s - 4 < NBLK:
                    emit_vstage(s - 4, 7)
                # loads after vstage7 so the ACT queue leads with copy_d7
                # (not two c1-load issues) and DVE's min_d7 is never the
                # first thing waiting on it
                if 3 <= s + 3 < NBLK:
                    emit_loads(s + 3)
                if s < NBLK:
                    emit_h(s, 1)
                if 0 <= s - 3 < NBLK:
                    emit_vstage(s - 3, 4)
                if s < NBLK:
                    emit_h(s, 2)
                if 0 <= s - 2 < NBLK:
                    emit_vstage(s - 2, 2)
                if s < NBLK:
                    emit_h(s, 3)
                if 0 <= s - 1 < NBLK:
                    emit_vstage(s - 1, 1)
                if s < NBLK:
                    emit_h(s, 4)

    nc.compile()
    return nc


def kernel(img: np.ndarray) -> np.ndarray:
    assert img.shape == (16, 3, 1024, 1024) and img.dtype == np.float32
    if "nc" not in _cache:
        _cache["nc"] = _build()
    nc = _cache["nc"]
    in_maps = [{"img": np.ascontiguousarray(img[2 * k:2 * k + 2])}
               for k in range(N_CORES)]
    res = bass_utils.run_bass_kernel_spmd(
        nc, in_maps, core_ids=list(range(N_CORES)))
    outs = [np.asarray(r["out"]).astype(np.float32) for r in res.results]
    return np.concatenate(outs, axis=0)
